# revision 56
# baseline (speedup 1.0000x reference)
"""Trainium2 Bass kernel for nn_Kernel3D (Gaussian splat onto a 64x64x64x8 grid).

Math:  out[x,y,z,t] = sum_n bx[n,x] * by[n,y] * bz[n,z] * x[n,t]
where b?[n,g] = exp(-0.5*((g-mu)/s)^2) / sqrt(2*pi*s^2).

v6: x-slab sharding (8 planes/core).  Points are binned into NC z-bands with
boundaries shared across all 8 cores (pooled quantiles), so the per-chunk
z-windows stay tight (band + 2*C*sigma) instead of drifting apart per core.
Per chunk the accumulated matmul is
    out[(x y), (z t)] += P[n, (x y)]^T @ Q[n, (z0..z0+wz) t]

Engine split per chunk (measured-rate aware: last-dim-contiguous fp16 DVE
ops run ~2x faster than last-dim-broadcast ones):
  DVE   d = iota - mu' (3 section ops); e = d * (1/sigma); q = bz x xc
        (one direct op); p = bxr * by-midbcast (fast mode)
  ACT   u = Square(e); b_yz = Exp(-.5 u); bxr = broadcast-Exp over y
  PE    8 zero-init matmuls (DVFS warm-up) + WARM_MM fillers, then the
        per-chunk accumulation stream
Stages are software-pipelined: A(g+1) is emitted before C(g) so DVE works
on the next group while ACT finishes the current one.
"""

import os
import sys

import numpy as np

for _p in ("/opt/trn_rl_repo", "/root/.axon_site/_ro/trn_rl_repo"):
    if os.path.isdir(_p) and _p not in sys.path:
        sys.path.insert(0, _p)

N_CORES = 8
GX, GY, GZ, GT = 64, 64, 64, 8
XPER = GX // N_CORES
PPC = 128
FEAT = 16  # x[8], mu'[3], sigma[3], pad[2]

SIGMA_CUT = 2.75  # y/z window support cut
SEL_CUT = 2.5  # x-slab selection cut (exact err cost ~3e-4, saves a chunk)
WARM_MM = 12  # extra zero matmuls (512 cols each) after the 8 init matmuls
IOX, IOY, IOZ = -4, -32, -16  # iota section bases (centers fp16 mu' ulp)

_prog_cache = {}


_walrus_patched = False


def _patch_walrus():
    # Cap the compiler's semaphore allocation: the NEFF epilogue clears its
    # semaphore file one register per instruction, so a smaller allocation
    # shortens the fixed teardown chain.
    global _walrus_patched
    if _walrus_patched:
        return
    _walrus_patched = True
    from concourse import bass_utils as _bu

    _orig = _bu.run_command

    def _run(cmd, cwd=None, **kw):
        if cmd and "walrus_driver" in str(cmd[0]):
            cmd = list(cmd) + ["--max-sem-num=64"]
        return _orig(cmd, cwd=cwd, **kw)

    _bu.run_command = _run


def _build(n_chunks, z0s, z1s, g0s, WZ, ZS):
    import concourse.bass as bass
    import concourse.tile as tile
    from concourse import mybir
    from contextlib import ExitStack

    f32 = mybir.dt.float32
    f16 = mybir.dt.float16
    bf16 = mybir.dt.bfloat16
    AL = mybir.AluOpType
    ACTF = mybir.ActivationFunctionType
    C0 = float((2.0 * np.pi) ** -1.5)
    NC = n_chunks
    L = XPER + GY + WZ  # flat grid segments [x | y | zwin]
    ZO = XPER + GY  # z segment offset
    HW = (ZS * GT, (GZ - ZS) * GT)  # used cols per (m,h) bank
    OH = (0, 4 * HW[0])  # o_t / out column offset of each half block

    # per-chunk (half, zlo, zhi) matmul parts; last chunk touching each half
    parts = []
    for c in range(NC):
        pr = []
        for h in (0, 1):
            zlo = max(z0s[c], ZS if h else 0)
            zhi = min(z1s[c], GZ if h else ZS)
            if zhi > zlo:
                pr.append((h, zlo, zhi))
        parts.append(pr)
    last_touch = {
        h: max(c for c in range(NC) if any(p[0] == h for p in parts[c]))
        for h in (0, 1)
    }

    # group schedule: small leading groups to start the PE stream early,
    # fat steady-state groups to amortize per-op fixed costs
    sizes = []
    rem = NC
    for s in (1, 1, 3):
        if rem <= 0:
            break
        s = min(s, rem)
        sizes.append(s)
        rem -= s
    while rem > 0:
        s = min(5, rem)
        if rem - s == 0 and s > 3:
            s = s - 2
        sizes.append(s)
        rem -= s
    bounds = [0]
    for s in sizes:
        bounds.append(bounds[-1] + s)
    groups = list(zip(bounds[:-1], bounds[1:]))
    G = len(groups)

    nc = bass.Bass(use_seq_codegen=True)
    inp = nc.declare_dram_parameter("inp", [PPC, NC * FEAT], f32, isOutput=False)
    out = nc.declare_dram_parameter("out", [PPC, GZ * GT * 4], bf16, isOutput=True)

    with tile.TileContext(nc) as tc, ExitStack() as ctx:
        cpool = ctx.enter_context(tc.tile_pool(name="const", bufs=1))
        ppool = ctx.enter_context(tc.tile_pool(name="accp", bufs=1, space="PSUM"))

        # tiny memset -> dummy activation pulls the act table load to t=0
        z1_t = cpool.tile([PPC, 1], f16, name="z1_t")
        nc.gpsimd.memset(z1_t[:, :], 0.0)
        dummy_t = cpool.tile([PPC, 1], f16, name="dummy_t")
        nc.scalar.activation(dummy_t[:, :], z1_t[:, 0:1], ACTF.Exp, scale=-0.5)

        # input DMA on the sync queue, split so chunks 0-1 land early
        inp_t = cpool.tile([PPC, NC * FEAT], f32, name="inp_t")
        CS = min(2, NC) * FEAT
        nc.sync.dma_start(inp_t[:, 0:CS], inp[:, 0:CS])
        nc.sync.dma_start(inp_t[:, CS:], inp[:, CS:])
        pts3 = inp_t[:, :].rearrange("p (c f) -> p c f", f=FEAT)

        zero_t = cpool.tile([PPC, 640], f16, name="zero_t")
        nc.gpsimd.memset(zero_t[:, :], 0.0)

        # one f32 iota row [x: -4.. | y: -32.. | z: -16..] (centered mu')
        iota_t = cpool.tile([PPC, L], f32, name="iota_t")
        for (a, b, nseg, base) in (
            (0, XPER, XPER, IOX),
            (XPER, ZO, GY, IOY),
            (ZO, L, WZ, IOZ),
        ):
            nc.gpsimd.iota(
                iota_t[:, a:b],
                pattern=[[1, nseg]],
                base=base,
                channel_multiplier=0,
                allow_small_or_imprecise_dtypes=True,
            )

        # PSUM: 8 banks, bank (m, h) at cols (2m+h)*512; zero-matmul init
        # (doubles as PE DVFS warm-up), then WARM_MM extra zero matmuls.
        acc = ppool.tile([128, 8 * 512], f32, name="acc")
        for m in range(4):
            for h in (0, 1):
                b = 2 * m + h
                nc.tensor.matmul(
                    acc[:, b * 512 : b * 512 + HW[h]],
                    lhsT=zero_t[:, 0:128],
                    rhs=zero_t[:, 128 : 128 + HW[h]],
                    start=True,
                    stop=False,
                )
        for w in range(WARM_MM):
            b = w % 8
            hw = HW[b % 2]
            nc.tensor.matmul(
                acc[:, b * 512 : b * 512 + hw],
                lhsT=zero_t[:, 0:128],
                rhs=zero_t[:, 128 : 128 + hw],
                start=False,
                stop=False,
            )

        # per-point scalars
        inv_t = cpool.tile([PPC, NC, 3], f32, name="inv_t")
        C2 = min(2, NC)
        nc.vector.reciprocal(inv_t[:, 0:C2, :], pts3[:, 0:C2, 11:14])
        m1_t = cpool.tile([PPC, NC], f32, name="m1_t")
        m2_t = cpool.tile([PPC, NC], f32, name="m2_t")
        xc_t = cpool.tile([PPC, NC, GT], f16, name="xc_t")

        am_t = cpool.tile([PPC, NC, L], f16, name="am_t")

        def emit_am(c0, c1, on_act=False):
            n = c1 - c0
            for (a, b, w, col) in (
                (0, XPER, XPER, 0),
                (XPER, ZO, GY, 1),
                (ZO, L, WZ, 2),
            ):
                srcv = inv_t[:, c0:c1, col : col + 1].broadcast_to((PPC, n, w))
                if on_act:
                    nc.scalar.copy(am_t[:, c0:c1, a:b], srcv)
                else:
                    nc.vector.tensor_copy(am_t[:, c0:c1, a:b], srcv)

        def emit_xc(c0, c1):
            n = c1 - c0
            nc.gpsimd.tensor_tensor(
                m1_t[:, c0:c1], inv_t[:, c0:c1, 0], inv_t[:, c0:c1, 1], AL.mult
            )
            nc.gpsimd.tensor_tensor(
                m2_t[:, c0:c1], m1_t[:, c0:c1], inv_t[:, c0:c1, 2], AL.mult
            )
            nc.vector.tensor_tensor(
                xc_t[:, c0:c1, :],
                pts3[:, c0:c1, 0:GT],
                m2_t[:, c0:c1].unsqueeze(2).broadcast_to((PPC, n, GT)),
                AL.mult,
            )
        d_t = cpool.tile([PPC, NC, L], f16, name="d_t")
        e_t = cpool.tile([PPC, NC, L], f16, name="e_t")
        u_t = cpool.tile([PPC, NC, L], f16, name="u_t")
        byz_t = cpool.tile([PPC, NC, GY + WZ], f16, name="byz_t")
        bzr_t = cpool.tile([PPC, NC, WZ, GT], f16, name="bzr_t")
        bxr_t = cpool.tile([PPC, NC, XPER, GY], f16, name="bxr_t")
        p_t = cpool.tile([PPC, NC, XPER, GY], f16, name="p_t")
        q_t = cpool.tile([PPC, NC, WZ, GT], f16, name="q_t")
        pf = p_t[:, :, :, :].rearrange("p c a b -> p c (a b)")
        qf = q_t[:, :, :, :].rearrange("p c a b -> p c (a b)")
        o_t = cpool.tile([128, GZ * GT * 4], bf16, name="o_t")

        def emit_evac(h):
            # evacuate the 4 (m,h) banks into a contiguous per-half block,
            # then fat-descriptor DMAs (half 0 copies entirely on ACT to
            # keep DVE clear for the build pipeline)
            W = HW[h]
            for m in range(4):
                b = 2 * m + h
                dst = o_t[:, OH[h] + m * W : OH[h] + (m + 1) * W]
                if m % 2 == 0:
                    nc.scalar.mul(dst, acc[:, b * 512 : b * 512 + W], C0)
                else:
                    nc.vector.tensor_scalar(
                        dst, acc[:, b * 512 : b * 512 + W], C0, None, AL.mult
                    )
                if h == 1 and m == 1:
                    c2 = slice(OH[1], OH[1] + 2 * W)
                    nc.scalar.dma_start(out[:, c2], o_t[:, c2])
            if h == 0:
                cols = slice(OH[0], OH[0] + 4 * W)
                nc.sync.dma_start(out[:, cols], o_t[:, cols])
            else:
                c2 = slice(OH[1] + 2 * W, OH[1] + 4 * W)
                nc.sync.dma_start(out[:, c2], o_t[:, c2])

        def emit_d(g):
            c0, c1 = groups[g]
            n = c1 - c0
            # GPSIMD: d = iota - mu' (3 small section ops, off DVE/ACT)
            for (a, b, w, col) in (
                (0, XPER, XPER, 0),
                (XPER, ZO, GY, 1),
                (ZO, L, WZ, 2),
            ):
                nc.gpsimd.tensor_tensor(
                    d_t[:, c0:c1, a:b],
                    iota_t[:, a:b].unsqueeze(1).broadcast_to((PPC, n, w)),
                    pts3[:, c0:c1, 8 + col : 9 + col].broadcast_to((PPC, n, w)),
                    AL.subtract,
                )

        def emit_eub(g):
            c0, c1 = groups[g]
            n = c1 - c0
            # DVE fast ops: e = d/sigma; u = e^2
            nc.vector.tensor_tensor(
                e_t[:, c0:c1, :], d_t[:, c0:c1, :], am_t[:, c0:c1, :], AL.mult
            )
            nc.vector.tensor_tensor(
                u_t[:, c0:c1, :], e_t[:, c0:c1, :], e_t[:, c0:c1, :], AL.mult
            )
            # ACT: compact exp (y,z) and bxr (broadcast-exp along y)
            nc.scalar.activation(
                byz_t[:, c0:c1, :], u_t[:, c0:c1, XPER:L], ACTF.Exp, scale=-0.5
            )
            nc.scalar.activation(
                bxr_t[:, c0:c1, :, :],
                u_t[:, c0:c1, 0:XPER].unsqueeze(3).broadcast_to((PPC, n, XPER, GY)),
                ACTF.Exp,
                scale=-0.5,
            )

        def emit_bzr(g):
            # bz replicated over t, produced one group ahead on DVE so it
            # fills DVE idle time while ACT runs the current group's exps
            c0, c1 = groups[g]
            n = c1 - c0
            nc.vector.tensor_copy(
                bzr_t[:, c0:c1, :, :],
                byz_t[:, c0:c1, GY : GY + WZ]
                .unsqueeze(3)
                .broadcast_to((PPC, n, WZ, GT)),
            )

        def emit_c(g):
            c0, c1 = groups[g]
            n = c1 - c0
            nc.vector.tensor_tensor(
                q_t[:, c0:c1, :, :],
                bzr_t[:, c0:c1, :, :],
                xc_t[:, c0:c1, :].unsqueeze(2).broadcast_to((PPC, n, WZ, GT)),
                AL.mult,
            )
            nc.vector.tensor_tensor(
                p_t[:, c0:c1, :, :],
                bxr_t[:, c0:c1, :, :],
                byz_t[:, c0:c1, 0:GY].unsqueeze(2).broadcast_to((PPC, n, XPER, GY)),
                AL.mult,
            )

        def emit_mm(g):
            c0, c1 = groups[g]
            for c in range(c0, c1):
                for (h, zlo, zhi) in parts[c]:
                    s = zlo - g0s[c]
                    w = zhi - zlo
                    zb = ZS if h else 0
                    stop = c == last_touch[h]
                    for m in range(4):
                        b = 2 * m + h
                        nc.tensor.matmul(
                            acc[
                                :,
                                b * 512 + (zlo - zb) * GT : b * 512
                                + (zhi - zb) * GT,
                            ],
                            lhsT=pf[:, c, m * 128 : (m + 1) * 128],
                            rhs=qf[:, c, s * GT : (s + w) * GT],
                            start=False,
                            stop=stop and m == 3,
                        )
                if c == last_touch[0]:
                    emit_evac(0)

        emit_d(0)
        if G > 1:
            emit_d(1)
        c01 = groups[1][1] if G > 1 else NC
        emit_xc(0, C2)
        emit_am(0, min(c01, C2))
        emit_eub(0)
        emit_bzr(0)
        for g in range(1, G):
            if g + 1 < G:
                emit_d(g + 1)
            emit_eub(g)
            if g == 1 and NC > C2:
                nc.vector.reciprocal(inv_t[:, C2:, :], pts3[:, C2:, 11:14])
                emit_am(C2, NC)
                emit_xc(C2, NC)
            emit_c(g - 1)
            emit_bzr(g)
            emit_mm(g - 1)
        emit_c(G - 1)
        emit_mm(G - 1)

        emit_evac(1)

    _split_multi_waits(nc, mybir)
    return nc


def _split_multi_waits(nc, mybir):
    k = 0
    for bb in nc.m.functions[0].blocks:
        new = []
        for inst in bb.instructions:
            si = inst.sync_info
            if si is not None and si.on_wait and len(si.on_wait) > 1:
                for w in si.on_wait[:-1]:
                    wi = mybir.InstEventSemaphore(name=f"wsplit_{k}", ins=[], outs=[])
                    k += 1
                    wi.engine = inst.engine
                    wi.sync_info = mybir.SyncInfo(on_wait=[w], on_update=[])
                    nc.register_instruction(wi)
                    new.append(wi)
                inst.sync_info = mybir.SyncInfo(
                    on_wait=[si.on_wait[-1]], on_update=si.on_update
                )
            new.append(inst)
        bb.instructions[:] = new


def _get_prog(n_chunks, z0s, z1s, g0s, WZ, ZS):
    key = (n_chunks, tuple(z0s), tuple(z1s), tuple(g0s), WZ, ZS, WARM_MM, "v48")
    if key not in _prog_cache:
        _prog_cache[key] = _build(n_chunks, z0s, z1s, g0s, WZ, ZS)
    return _prog_cache[key]


def _pack_points(x, mu, sigma, chunk_of, n_chunks, z0s, wzs, g0s, core):
    # mu is transformed to mu': iota-section offsets folded in so the device
    # computes d = iota_l - mu' directly.
    feat = np.zeros((n_chunks, PPC, FEAT), np.float32)
    feat[:, :, 11:14] = 1.0
    for c in range(n_chunks):
        feat[c, :, 8] = 4.0 + IOX  # pads: benign mu', zero x -> zero contrib
        feat[c, :, 9] = GY / 2.0 + IOY
        feat[c, :, 10] = wzs[c] / 2.0 + (z0s[c] - g0s[c]) + IOZ
    fill = np.zeros(n_chunks, np.int64)
    for i in range(x.shape[0]):
        c = chunk_of[i]
        j = fill[c]
        fill[c] = j + 1
        feat[c, j, 0:8] = x[i]
        feat[c, j, 8] = mu[i, 0] - XPER * core + IOX
        feat[c, j, 9] = mu[i, 1] + IOY
        feat[c, j, 10] = mu[i, 2] - g0s[c] + IOZ
        feat[c, j, 11:14] = sigma[i]
    return feat.transpose(1, 0, 2).reshape(PPC, n_chunks * FEAT)


def _band_chunks(sel_mu_z, pooled_mu_z):
    """Shared z-band boundaries (pooled quantiles); per core, points are
    assigned to bands in sorted order with a 128 cap enforced by cumulative
    clipping (order-preserving spill into neighbor bands)."""
    max_sel = max(len(s) for s in sel_mu_z)
    n_chunks = max(1, int(np.ceil(max_sel / 124.0)))
    while True:
        qs = np.quantile(pooled_mu_z, np.linspace(0, 1, n_chunks + 1)[1:-1])
        ok = True
        assign = []
        for z in sel_mu_z:
            n = len(z)
            if n > n_chunks * PPC:
                ok = False
                break
            order = np.argsort(z, kind="stable")
            zs = z[order]
            # target cumulative counts per band, then enforce the 128 cap:
            # forward pass caps each step, backward pass lifts to reach n
            cum = np.searchsorted(zs, qs).astype(np.int64)
            cum = np.append(cum, n)
            cum = np.maximum.accumulate(cum)
            prev = 0
            for b in range(n_chunks):
                cum[b] = min(cum[b], prev + PPC)
                prev = cum[b]
            cum[n_chunks - 1] = n
            for b in range(n_chunks - 2, -1, -1):
                cum[b] = max(cum[b], cum[b + 1] - PPC)
            chunk_sorted = np.repeat(
                np.arange(n_chunks), np.diff(np.append(0, cum))
            )
            chunk_of = np.empty(n, np.int64)
            chunk_of[order] = chunk_sorted
            assign.append(chunk_of)
        if ok:
            return n_chunks, assign
        n_chunks += 1


def _prepare(x, mu, sigma):
    n = x.shape[0]
    C = SIGMA_CUT
    sel = []
    for c in range(N_CORES):
        lo, hi = c * XPER, c * XPER + XPER - 1
        d = np.maximum.reduce([lo - mu[:, 0], mu[:, 0] - hi, np.zeros(n, np.float32)])
        idx = np.nonzero(d <= SEL_CUT * sigma[:, 0])[0]
        sel.append(idx)
    pooled = np.concatenate([mu[idx, 2] for idx in sel])
    n_chunks, assign = _band_chunks([mu[idx, 2] for idx in sel], pooled)

    z0s, z1s = [], []
    for c in range(n_chunks):
        zlo, zhi = GZ, 0
        for k in range(N_CORES):
            idx = sel[k][assign[k] == c]
            if len(idx):
                zlo = min(zlo, np.min(mu[idx, 2] - C * sigma[idx, 2]))
                zhi = max(zhi, np.max(mu[idx, 2] + C * sigma[idx, 2]))
        z0 = max(0, int(np.floor(zlo)))
        z1 = min(GZ, int(np.ceil(zhi)))
        if z1 <= z0:
            z0, z1 = 0, 1
        z0s.append(z0)
        z1s.append(z1)
    wzs = [z1s[c] - z0s[c] for c in range(n_chunks)]
    WZ = max(wzs)
    g0s = [z0s[c] if z0s[c] + WZ <= GZ else GZ - WZ for c in range(n_chunks)]
    # z-split: pick the largest ZS such that no chunk after ~70% of the
    # stream touches z < ZS, so most of the output DMAs out mid-stream
    c_split = max(1, min(n_chunks - 1, int(round(0.7 * n_chunks))))
    ZS = int(min(63, max(1, min(z0s[c] for c in range(c_split, n_chunks)))))

    in_maps = []
    for k in range(N_CORES):
        idx = sel[k]
        inp = _pack_points(
            x[idx], mu[idx], sigma[idx], assign[k], n_chunks, z0s, wzs, g0s, k
        )
        in_maps.append({"inp": np.ascontiguousarray(inp)})
    return in_maps, n_chunks, z0s, z1s, g0s, WZ, ZS


def _assemble(results, ZS):
    full = np.zeros((N_CORES, 512, GZ * GT), np.float32)
    W0, W1 = ZS * GT, (GZ - ZS) * GT
    for k in range(N_CORES):
        o = np.asarray(results[k]["out"]).astype(np.float32)  # [128, 2048] bf16
        for m in range(4):
            full[k, m * 128 : (m + 1) * 128, 0:W0] = o[:, m * W0 : (m + 1) * W0]
            full[k, m * 128 : (m + 1) * 128, W0:] = o[
                :, 4 * W0 + m * W1 : 4 * W0 + (m + 1) * W1
            ]
    o = full.reshape(N_CORES, XPER, GY, GZ, GT)
    return np.ascontiguousarray(o.reshape(GX, GY, GZ, GT))


def run(x, mu, sigma, trace=False, **spmd_kwargs):
    from concourse.bass_utils import run_bass_kernel_spmd

    x = np.asarray(x, np.float32)
    mu = np.asarray(mu, np.float32)
    sigma = np.asarray(sigma, np.float32)
    in_maps, n_chunks, z0s, z1s, g0s, WZ, ZS = _prepare(x, mu, sigma)
    nc = _get_prog(n_chunks, z0s, z1s, g0s, WZ, ZS)
    res = run_bass_kernel_spmd(
        nc, in_maps, list(range(N_CORES)), trace=trace, **spmd_kwargs
    )
    return _assemble(res.results, ZS), res


def kernel(x, mu, sigma):
    out, _ = run(x, mu, sigma)
    return out


# revision 57
# speedup vs baseline: 1.0152x; 1.0152x over previous
"""Trainium2 Bass kernel for nn_Kernel3D (Gaussian splat onto a 64x64x64x8 grid).

Math:  out[x,y,z,t] = sum_n bx[n,x] * by[n,y] * bz[n,z] * x[n,t]
where b?[n,g] = exp(-0.5*((g-mu)/s)^2) / sqrt(2*pi*s^2).

v6: x-slab sharding (8 planes/core).  Points are binned into NC z-bands with
boundaries shared across all 8 cores (pooled quantiles), so the per-chunk
z-windows stay tight (band + 2*C*sigma) instead of drifting apart per core.
Per chunk the accumulated matmul is
    out[(x y), (z t)] += P[n, (x y)]^T @ Q[n, (z0..z0+wz) t]

Engine split per chunk (measured-rate aware: last-dim-contiguous fp16 DVE
ops run ~2x faster than last-dim-broadcast ones):
  DVE   d = iota - mu' (3 section ops); e = d * (1/sigma); q = bz x xc
        (one direct op); p = bxr * by-midbcast (fast mode)
  ACT   u = Square(e); b_yz = Exp(-.5 u); bxr = broadcast-Exp over y
  PE    8 zero-init matmuls (DVFS warm-up) + WARM_MM fillers, then the
        per-chunk accumulation stream
Stages are software-pipelined: A(g+1) is emitted before C(g) so DVE works
on the next group while ACT finishes the current one.
"""

import os
import sys

import numpy as np

for _p in ("/opt/trn_rl_repo", "/root/.axon_site/_ro/trn_rl_repo"):
    if os.path.isdir(_p) and _p not in sys.path:
        sys.path.insert(0, _p)

N_CORES = 8
GX, GY, GZ, GT = 64, 64, 64, 8
XPER = GX // N_CORES
PPC = 128
FEAT = 16  # x[8], mu'[3], sigma[3], pad[2]

SIGMA_CUT = 2.75  # y/z window support cut
SEL_CUT = 2.5  # x-slab selection cut (exact err cost ~3e-4, saves a chunk)
WARM_MM = 6  # extra zero matmuls (512 cols each) after the 8 init matmuls
IOX, IOY, IOZ = -4, -32, -16  # iota section bases (centers fp16 mu' ulp)

_prog_cache = {}


_walrus_patched = False


def _patch_walrus():
    # Cap the compiler's semaphore allocation: the NEFF epilogue clears its
    # semaphore file one register per instruction, so a smaller allocation
    # shortens the fixed teardown chain.
    global _walrus_patched
    if _walrus_patched:
        return
    _walrus_patched = True
    from concourse import bass_utils as _bu

    _orig = _bu.run_command

    def _run(cmd, cwd=None, **kw):
        if cmd and "walrus_driver" in str(cmd[0]):
            cmd = list(cmd) + ["--max-sem-num=64"]
        return _orig(cmd, cwd=cwd, **kw)

    _bu.run_command = _run


def _build(n_chunks, z0s, z1s, g0s, WZ, ZS):
    import concourse.bass as bass
    import concourse.tile as tile
    from concourse import mybir
    from contextlib import ExitStack

    f32 = mybir.dt.float32
    f16 = mybir.dt.float16
    bf16 = mybir.dt.bfloat16
    AL = mybir.AluOpType
    ACTF = mybir.ActivationFunctionType
    C0 = float((2.0 * np.pi) ** -1.5)
    NC = n_chunks
    L = XPER + GY + WZ  # flat grid segments [x | y | zwin]
    ZO = XPER + GY  # z segment offset
    HW = (ZS * GT, (GZ - ZS) * GT)  # used cols per (m,h) bank
    OH = (0, 4 * HW[0])  # o_t / out column offset of each half block

    # per-chunk (half, zlo, zhi) matmul parts; last chunk touching each half
    parts = []
    for c in range(NC):
        pr = []
        for h in (0, 1):
            zlo = max(z0s[c], ZS if h else 0)
            zhi = min(z1s[c], GZ if h else ZS)
            if zhi > zlo:
                pr.append((h, zlo, zhi))
        parts.append(pr)
    last_touch = {
        h: max(c for c in range(NC) if any(p[0] == h for p in parts[c]))
        for h in (0, 1)
    }

    # group schedule: small leading groups to start the PE stream early,
    # fat steady-state groups to amortize per-op fixed costs
    sizes = []
    rem = NC
    for s in (1, 1, 3):
        if rem <= 0:
            break
        s = min(s, rem)
        sizes.append(s)
        rem -= s
    while rem > 0:
        s = min(5, rem)
        if rem - s == 0 and s > 3:
            s = s - 2
        sizes.append(s)
        rem -= s
    bounds = [0]
    for s in sizes:
        bounds.append(bounds[-1] + s)
    groups = list(zip(bounds[:-1], bounds[1:]))
    G = len(groups)

    nc = bass.Bass(use_seq_codegen=True)
    inp = nc.declare_dram_parameter("inp", [PPC, NC * FEAT], f32, isOutput=False)
    out = nc.declare_dram_parameter("out", [PPC, GZ * GT * 4], bf16, isOutput=True)

    with tile.TileContext(nc) as tc, ExitStack() as ctx:
        cpool = ctx.enter_context(tc.tile_pool(name="const", bufs=1))
        ppool = ctx.enter_context(tc.tile_pool(name="accp", bufs=1, space="PSUM"))

        # tiny memset -> dummy activation pulls the act table load to t=0
        z1_t = cpool.tile([PPC, 1], f16, name="z1_t")
        nc.gpsimd.memset(z1_t[:, :], 0.0)
        dummy_t = cpool.tile([PPC, 1], f16, name="dummy_t")
        nc.scalar.activation(dummy_t[:, :], z1_t[:, 0:1], ACTF.Exp, scale=-0.5)

        # input DMA on the sync queue, split so chunks 0-1 land early
        inp_t = cpool.tile([PPC, NC * FEAT], f32, name="inp_t")
        CS = min(2, NC) * FEAT
        nc.sync.dma_start(inp_t[:, 0:CS], inp[:, 0:CS])
        nc.sync.dma_start(inp_t[:, CS:], inp[:, CS:])
        pts3 = inp_t[:, :].rearrange("p (c f) -> p c f", f=FEAT)

        zero_t = cpool.tile([PPC, 640], f16, name="zero_t")
        nc.gpsimd.memset(zero_t[:, :], 0.0)

        # one f32 iota row [x: -4.. | y: -32.. | z: -16..] (centered mu')
        iota_t = cpool.tile([PPC, L], f32, name="iota_t")
        for (a, b, nseg, base) in (
            (0, XPER, XPER, IOX),
            (XPER, ZO, GY, IOY),
            (ZO, L, WZ, IOZ),
        ):
            nc.gpsimd.iota(
                iota_t[:, a:b],
                pattern=[[1, nseg]],
                base=base,
                channel_multiplier=0,
                allow_small_or_imprecise_dtypes=True,
            )

        # PSUM: 8 banks, bank (m, h) at cols (2m+h)*512; zero-matmul init
        # (doubles as PE DVFS warm-up), then WARM_MM extra zero matmuls.
        acc = ppool.tile([128, 8 * 512], f32, name="acc")
        for m in range(4):
            for h in (0, 1):
                b = 2 * m + h
                nc.tensor.matmul(
                    acc[:, b * 512 : b * 512 + HW[h]],
                    lhsT=zero_t[:, 0:128],
                    rhs=zero_t[:, 128 : 128 + HW[h]],
                    start=True,
                    stop=False,
                )
        for w in range(WARM_MM):
            b = w % 8
            hw = HW[b % 2]
            nc.tensor.matmul(
                acc[:, b * 512 : b * 512 + hw],
                lhsT=zero_t[:, 0:128],
                rhs=zero_t[:, 128 : 128 + hw],
                start=False,
                stop=False,
            )

        # per-point scalars
        inv_t = cpool.tile([PPC, NC, 3], f32, name="inv_t")
        C2 = min(2, NC)
        nc.vector.reciprocal(inv_t[:, 0:C2, :], pts3[:, 0:C2, 11:14])
        m1_t = cpool.tile([PPC, NC], f32, name="m1_t")
        m2_t = cpool.tile([PPC, NC], f32, name="m2_t")
        xc_t = cpool.tile([PPC, NC, GT], f16, name="xc_t")

        am_t = cpool.tile([PPC, NC, L], f16, name="am_t")

        def emit_am(c0, c1, on_act=False):
            n = c1 - c0
            for (a, b, w, col) in (
                (0, XPER, XPER, 0),
                (XPER, ZO, GY, 1),
                (ZO, L, WZ, 2),
            ):
                srcv = inv_t[:, c0:c1, col : col + 1].broadcast_to((PPC, n, w))
                if on_act:
                    nc.scalar.copy(am_t[:, c0:c1, a:b], srcv)
                else:
                    nc.vector.tensor_copy(am_t[:, c0:c1, a:b], srcv)

        def emit_xc(c0, c1):
            n = c1 - c0
            nc.gpsimd.tensor_tensor(
                m1_t[:, c0:c1], inv_t[:, c0:c1, 0], inv_t[:, c0:c1, 1], AL.mult
            )
            nc.gpsimd.tensor_tensor(
                m2_t[:, c0:c1], m1_t[:, c0:c1], inv_t[:, c0:c1, 2], AL.mult
            )
            nc.vector.tensor_tensor(
                xc_t[:, c0:c1, :],
                pts3[:, c0:c1, 0:GT],
                m2_t[:, c0:c1].unsqueeze(2).broadcast_to((PPC, n, GT)),
                AL.mult,
            )
        d_t = cpool.tile([PPC, NC, L], f16, name="d_t")
        e_t = cpool.tile([PPC, NC, L], f16, name="e_t")
        u_t = cpool.tile([PPC, NC, L], f16, name="u_t")
        byz_t = cpool.tile([PPC, NC, GY + WZ], f16, name="byz_t")
        bzr_t = cpool.tile([PPC, NC, WZ, GT], f16, name="bzr_t")
        bxr_t = cpool.tile([PPC, NC, XPER, GY], f16, name="bxr_t")
        p_t = cpool.tile([PPC, NC, XPER, GY], f16, name="p_t")
        q_t = cpool.tile([PPC, NC, WZ, GT], f16, name="q_t")
        pf = p_t[:, :, :, :].rearrange("p c a b -> p c (a b)")
        qf = q_t[:, :, :, :].rearrange("p c a b -> p c (a b)")
        o_t = cpool.tile([128, GZ * GT * 4], bf16, name="o_t")

        def emit_evac(h):
            # evacuate the 4 (m,h) banks into a contiguous per-half block,
            # then fat-descriptor DMAs (half 0 copies entirely on ACT to
            # keep DVE clear for the build pipeline)
            W = HW[h]
            for m in range(4):
                b = 2 * m + h
                dst = o_t[:, OH[h] + m * W : OH[h] + (m + 1) * W]
                if m % 2 == 0:
                    nc.scalar.mul(dst, acc[:, b * 512 : b * 512 + W], C0)
                else:
                    nc.vector.tensor_scalar(
                        dst, acc[:, b * 512 : b * 512 + W], C0, None, AL.mult
                    )
                if h == 1 and m == 1:
                    c2 = slice(OH[1], OH[1] + 2 * W)
                    nc.scalar.dma_start(out[:, c2], o_t[:, c2])
            if h == 0:
                cols = slice(OH[0], OH[0] + 4 * W)
                nc.sync.dma_start(out[:, cols], o_t[:, cols])
            else:
                c2 = slice(OH[1] + 2 * W, OH[1] + 4 * W)
                nc.sync.dma_start(out[:, c2], o_t[:, c2])

        def emit_d(g):
            c0, c1 = groups[g]
            n = c1 - c0
            # GPSIMD: d = iota - mu' (3 small section ops, off DVE/ACT)
            for (a, b, w, col) in (
                (0, XPER, XPER, 0),
                (XPER, ZO, GY, 1),
                (ZO, L, WZ, 2),
            ):
                nc.gpsimd.tensor_tensor(
                    d_t[:, c0:c1, a:b],
                    iota_t[:, a:b].unsqueeze(1).broadcast_to((PPC, n, w)),
                    pts3[:, c0:c1, 8 + col : 9 + col].broadcast_to((PPC, n, w)),
                    AL.subtract,
                )

        def emit_eub(g):
            c0, c1 = groups[g]
            n = c1 - c0
            # DVE fast ops: e = d/sigma; u = e^2
            nc.vector.tensor_tensor(
                e_t[:, c0:c1, :], d_t[:, c0:c1, :], am_t[:, c0:c1, :], AL.mult
            )
            nc.vector.tensor_tensor(
                u_t[:, c0:c1, :], e_t[:, c0:c1, :], e_t[:, c0:c1, :], AL.mult
            )
            # ACT: compact exp (y,z) and bxr (broadcast-exp along y)
            nc.scalar.activation(
                byz_t[:, c0:c1, :], u_t[:, c0:c1, XPER:L], ACTF.Exp, scale=-0.5
            )
            nc.scalar.activation(
                bxr_t[:, c0:c1, :, :],
                u_t[:, c0:c1, 0:XPER].unsqueeze(3).broadcast_to((PPC, n, XPER, GY)),
                ACTF.Exp,
                scale=-0.5,
            )

        def emit_bzr(g):
            # bz replicated over t, produced one group ahead on DVE so it
            # fills DVE idle time while ACT runs the current group's exps
            c0, c1 = groups[g]
            n = c1 - c0
            nc.vector.tensor_copy(
                bzr_t[:, c0:c1, :, :],
                byz_t[:, c0:c1, GY : GY + WZ]
                .unsqueeze(3)
                .broadcast_to((PPC, n, WZ, GT)),
            )

        def emit_c(g):
            c0, c1 = groups[g]
            n = c1 - c0
            nc.vector.tensor_tensor(
                q_t[:, c0:c1, :, :],
                bzr_t[:, c0:c1, :, :],
                xc_t[:, c0:c1, :].unsqueeze(2).broadcast_to((PPC, n, WZ, GT)),
                AL.mult,
            )
            nc.vector.tensor_tensor(
                p_t[:, c0:c1, :, :],
                bxr_t[:, c0:c1, :, :],
                byz_t[:, c0:c1, 0:GY].unsqueeze(2).broadcast_to((PPC, n, XPER, GY)),
                AL.mult,
            )

        def emit_mm(g):
            c0, c1 = groups[g]
            for c in range(c0, c1):
                for (h, zlo, zhi) in parts[c]:
                    s = zlo - g0s[c]
                    w = zhi - zlo
                    zb = ZS if h else 0
                    stop = c == last_touch[h]
                    for m in range(4):
                        b = 2 * m + h
                        nc.tensor.matmul(
                            acc[
                                :,
                                b * 512 + (zlo - zb) * GT : b * 512
                                + (zhi - zb) * GT,
                            ],
                            lhsT=pf[:, c, m * 128 : (m + 1) * 128],
                            rhs=qf[:, c, s * GT : (s + w) * GT],
                            start=False,
                            stop=stop and m == 3,
                        )
                if c == last_touch[0]:
                    emit_evac(0)

        emit_d(0)
        if G > 1:
            emit_d(1)
        c01 = groups[1][1] if G > 1 else NC
        emit_xc(0, C2)
        emit_am(0, min(c01, C2))
        emit_eub(0)
        emit_bzr(0)
        for g in range(1, G):
            if g + 1 < G:
                emit_d(g + 1)
            emit_eub(g)
            if g == 1 and NC > C2:
                nc.vector.reciprocal(inv_t[:, C2:, :], pts3[:, C2:, 11:14])
                emit_am(C2, NC)
                emit_xc(C2, NC)
            emit_c(g - 1)
            emit_bzr(g)
            emit_mm(g - 1)
        emit_c(G - 1)
        emit_mm(G - 1)

        emit_evac(1)

    _split_multi_waits(nc, mybir)
    return nc


def _split_multi_waits(nc, mybir):
    k = 0
    for bb in nc.m.functions[0].blocks:
        new = []
        for inst in bb.instructions:
            si = inst.sync_info
            if si is not None and si.on_wait and len(si.on_wait) > 1:
                for w in si.on_wait[:-1]:
                    wi = mybir.InstEventSemaphore(name=f"wsplit_{k}", ins=[], outs=[])
                    k += 1
                    wi.engine = inst.engine
                    wi.sync_info = mybir.SyncInfo(on_wait=[w], on_update=[])
                    nc.register_instruction(wi)
                    new.append(wi)
                inst.sync_info = mybir.SyncInfo(
                    on_wait=[si.on_wait[-1]], on_update=si.on_update
                )
            new.append(inst)
        bb.instructions[:] = new


def _get_prog(n_chunks, z0s, z1s, g0s, WZ, ZS):
    key = (n_chunks, tuple(z0s), tuple(z1s), tuple(g0s), WZ, ZS, WARM_MM, "v49")
    if key not in _prog_cache:
        _prog_cache[key] = _build(n_chunks, z0s, z1s, g0s, WZ, ZS)
    return _prog_cache[key]


def _pack_points(x, mu, sigma, chunk_of, n_chunks, z0s, wzs, g0s, core):
    # mu is transformed to mu': iota-section offsets folded in so the device
    # computes d = iota_l - mu' directly.
    feat = np.zeros((n_chunks, PPC, FEAT), np.float32)
    feat[:, :, 11:14] = 1.0
    for c in range(n_chunks):
        feat[c, :, 8] = 4.0 + IOX  # pads: benign mu', zero x -> zero contrib
        feat[c, :, 9] = GY / 2.0 + IOY
        feat[c, :, 10] = wzs[c] / 2.0 + (z0s[c] - g0s[c]) + IOZ
    fill = np.zeros(n_chunks, np.int64)
    for i in range(x.shape[0]):
        c = chunk_of[i]
        j = fill[c]
        fill[c] = j + 1
        feat[c, j, 0:8] = x[i]
        feat[c, j, 8] = mu[i, 0] - XPER * core + IOX
        feat[c, j, 9] = mu[i, 1] + IOY
        feat[c, j, 10] = mu[i, 2] - g0s[c] + IOZ
        feat[c, j, 11:14] = sigma[i]
    return feat.transpose(1, 0, 2).reshape(PPC, n_chunks * FEAT)


def _band_chunks(sel_mu_z, pooled_mu_z):
    """Shared z-band boundaries (pooled quantiles); per core, points are
    assigned to bands in sorted order with a 128 cap enforced by cumulative
    clipping (order-preserving spill into neighbor bands)."""
    max_sel = max(len(s) for s in sel_mu_z)
    n_chunks = max(1, int(np.ceil(max_sel / 124.0)))
    while True:
        qs = np.quantile(pooled_mu_z, np.linspace(0, 1, n_chunks + 1)[1:-1])
        ok = True
        assign = []
        for z in sel_mu_z:
            n = len(z)
            if n > n_chunks * PPC:
                ok = False
                break
            order = np.argsort(z, kind="stable")
            zs = z[order]
            # target cumulative counts per band, then enforce the 128 cap:
            # forward pass caps each step, backward pass lifts to reach n
            cum = np.searchsorted(zs, qs).astype(np.int64)
            cum = np.append(cum, n)
            cum = np.maximum.accumulate(cum)
            prev = 0
            for b in range(n_chunks):
                cum[b] = min(cum[b], prev + PPC)
                prev = cum[b]
            cum[n_chunks - 1] = n
            for b in range(n_chunks - 2, -1, -1):
                cum[b] = max(cum[b], cum[b + 1] - PPC)
            chunk_sorted = np.repeat(
                np.arange(n_chunks), np.diff(np.append(0, cum))
            )
            chunk_of = np.empty(n, np.int64)
            chunk_of[order] = chunk_sorted
            assign.append(chunk_of)
        if ok:
            return n_chunks, assign
        n_chunks += 1


def _prepare(x, mu, sigma):
    n = x.shape[0]
    C = SIGMA_CUT
    sel = []
    for c in range(N_CORES):
        lo, hi = c * XPER, c * XPER + XPER - 1
        d = np.maximum.reduce([lo - mu[:, 0], mu[:, 0] - hi, np.zeros(n, np.float32)])
        idx = np.nonzero(d <= SEL_CUT * sigma[:, 0])[0]
        sel.append(idx)
    pooled = np.concatenate([mu[idx, 2] for idx in sel])
    n_chunks, assign = _band_chunks([mu[idx, 2] for idx in sel], pooled)

    z0s, z1s = [], []
    for c in range(n_chunks):
        zlo, zhi = GZ, 0
        for k in range(N_CORES):
            idx = sel[k][assign[k] == c]
            if len(idx):
                zlo = min(zlo, np.min(mu[idx, 2] - C * sigma[idx, 2]))
                zhi = max(zhi, np.max(mu[idx, 2] + C * sigma[idx, 2]))
        z0 = max(0, int(np.floor(zlo)))
        z1 = min(GZ, int(np.ceil(zhi)))
        if z1 <= z0:
            z0, z1 = 0, 1
        z0s.append(z0)
        z1s.append(z1)
    wzs = [z1s[c] - z0s[c] for c in range(n_chunks)]
    WZ = max(wzs)
    g0s = [z0s[c] if z0s[c] + WZ <= GZ else GZ - WZ for c in range(n_chunks)]
    # z-split: pick the largest ZS such that no chunk after ~70% of the
    # stream touches z < ZS, so most of the output DMAs out mid-stream
    c_split = max(1, min(n_chunks - 1, int(round(0.7 * n_chunks))))
    ZS = int(min(63, max(1, min(z0s[c] for c in range(c_split, n_chunks)))))

    in_maps = []
    for k in range(N_CORES):
        idx = sel[k]
        inp = _pack_points(
            x[idx], mu[idx], sigma[idx], assign[k], n_chunks, z0s, wzs, g0s, k
        )
        in_maps.append({"inp": np.ascontiguousarray(inp)})
    return in_maps, n_chunks, z0s, z1s, g0s, WZ, ZS


def _assemble(results, ZS):
    full = np.zeros((N_CORES, 512, GZ * GT), np.float32)
    W0, W1 = ZS * GT, (GZ - ZS) * GT
    for k in range(N_CORES):
        o = np.asarray(results[k]["out"]).astype(np.float32)  # [128, 2048] bf16
        for m in range(4):
            full[k, m * 128 : (m + 1) * 128, 0:W0] = o[:, m * W0 : (m + 1) * W0]
            full[k, m * 128 : (m + 1) * 128, W0:] = o[
                :, 4 * W0 + m * W1 : 4 * W0 + (m + 1) * W1
            ]
    o = full.reshape(N_CORES, XPER, GY, GZ, GT)
    return np.ascontiguousarray(o.reshape(GX, GY, GZ, GT))


def run(x, mu, sigma, trace=False, **spmd_kwargs):
    from concourse.bass_utils import run_bass_kernel_spmd

    x = np.asarray(x, np.float32)
    mu = np.asarray(mu, np.float32)
    sigma = np.asarray(sigma, np.float32)
    in_maps, n_chunks, z0s, z1s, g0s, WZ, ZS = _prepare(x, mu, sigma)
    nc = _get_prog(n_chunks, z0s, z1s, g0s, WZ, ZS)
    res = run_bass_kernel_spmd(
        nc, in_maps, list(range(N_CORES)), trace=trace, **spmd_kwargs
    )
    return _assemble(res.results, ZS), res


def kernel(x, mu, sigma):
    out, _ = run(x, mu, sigma)
    return out
